# revision 1
# baseline (speedup 1.0000x reference)
"""CrossGatedAttentionGate Trainium2 kernel (8 NeuronCores).

Sharding: core c = 4*b + i handles (branch i, batch b): both of that
branch's Mamba layers (g-layer i, x-layer 4+i), the branch conv block, a
partial of the combine 3x3 conv (reduced over the 4 same-batch cores with an
in-kernel AllReduce), and the final x*psi for its 64-channel slice.

Mamba scan: per state-index n (N=16), decay dA_n = exp(A[:,n]*dt) (ScalarE
for odd exponents, GPSIMD dA_prev*dA1 chain for even), dBx_n =
(2^14*dt*xc) * broadcast(B_n), recurrence via the HW tensor_tensor_scan,
y = sum_n C_n*h_n accumulated in two halves (DVE + GPSIMD). B/C rows are
partition-broadcast with one-hot stationary matmuls on the PE, evicted
PSUM->SBUF(f16) on ScalarE/DVE per a tunable pattern. Bulk tensors are f16
(tolerance ~2e-2 relative; the scan term of y is tiny vs Dp*xc, so f16 is
comfortably safe). The scan loop runs in two L/2 chunks chained through a
carry column so both jobs' working sets co-reside in SBUF and job 1's prep
overlaps job 0's scan loop.
"""
import numpy as np

B, HH, WW = 2, 64, 64
NB, C, L = 4, 64, 64 * 64
DIN, N, DTR, DCONV, FINT = 128, 16, 4, 4, 64
USCALE = 16384.0           # 2^14 rescale keeping dBx in f16 range
LC = 512                   # PSUM matmul piece
LH = L // 2                # scan-loop half chunk
BCP = 1024                 # B/C broadcast PSUM piece

# engine assignment knobs (env-overridable for A/B benchmarking)
import os as _os
EV_PATTERN = _os.environ.get("K_EV", "AAADAAAD")  # evictions: A=ScalarE, D=DVE
ACC_DVE_EVERY = int(_os.environ.get("K_ACC", "1"))  # n%K==K-1 -> DVE else GPSIMD
DA_GPS = _os.environ.get("K_DAGPS", "0") == "1"     # even-exponent decays on GPSIMD

_COMPILED = {}


def _build(collective=True, reps=1):
    import concourse.bass as bass
    import concourse.mybir as mybir
    import concourse.tile as tile
    from contextlib import ExitStack

    F16 = mybir.dt.float16
    F32 = mybir.dt.float32
    AF = mybir.ActivationFunctionType
    ALU = mybir.AluOpType

    nc = bass.Bass("TRN2", num_devices=8 if collective else 1, debug=False)
    di = {}

    def inp(name, shape, dt=F16):
        di[name] = nc.dram_tensor(name, shape, dt, kind="ExternalInput")
        return di[name]

    inp("seq_g", (C, L)); inp("seq_x", (C, L))
    inp("xslice", (C, L), F32)
    inp("inwT", (C, 4 * DIN)); inp("cwdiag", (DIN, 2 * DCONV * DIN))
    inp("cb", (DIN, 2), F32)
    inp("xpwT", (DIN, 2 * (DTR + 2 * N))); inp("dtwT", (DTR, 2 * DIN))
    inp("dtb", (DIN, 2), F32); inp("Acol", (DIN, 2 * N), F32)
    inp("Dp", (DIN, 2), F32); inp("owT", (DIN, 2 * C))
    inp("ones1", (1, DIN)); inp("onehotBC", (2 * N, 2 * N * DIN))
    inp("dwdiag", (DIN, 9 * C)); inp("dwbias", (DIN, 1), F32)
    inp("pdwdiag", (DIN, 9 * C)); inp("pdwbias", (C, 1), F32)
    inp("p1wT", (C, FINT)); inp("p1bias", (FINT, 1), F32)
    inp("c3stat", (C, 9 * FINT)); inp("c3bias", (FINT, 1), F32)
    inp("c1w", (FINT, 1)); inp("c1bias", (1, 1), F32)
    out_d = nc.dram_tensor("outsl", (C, L), F32, kind="ExternalOutput")

    NCH = L // LC     # 8 pieces of 512
    E = DTR + 2 * N   # 36

    with ExitStack() as ctx:
        tc = ctx.enter_context(tile.TileContext(nc))
        wp = ctx.enter_context(tc.tile_pool(name="wp", bufs=1))
        big = ctx.enter_context(tc.tile_pool(name="big", bufs=1))

        def wload(pool, name, shape, dt=F16):
            t = pool.tile(list(shape), dt, tag=name)
            nc.sync.dma_start(t[:], di[name].ap())
            return t

        inwT = wload(wp, "inwT", (C, 4 * DIN))
        cwdiag = wload(wp, "cwdiag", (DIN, 2 * DCONV * DIN))
        cb = wload(wp, "cb", (DIN, 2), F32)
        xpwT = wload(wp, "xpwT", (DIN, 2 * E))
        dtw36 = wp.tile([2 * N + DTR, 2 * DIN], F16, tag="dtw36")
        nc.sync.dma_start(dtw36[2 * N:2 * N + DTR, :], di["dtwT"].ap())
        dtb = wload(wp, "dtb", (DIN, 2), F32)
        Acol = wload(wp, "Acol", (DIN, 2 * N), F32)
        Dpw = wload(wp, "Dp", (DIN, 2), F32)
        owT = wload(wp, "owT", (DIN, 2 * C))
        ones1 = wload(wp, "ones1", (1, DIN))
        onehotBC = wload(wp, "onehotBC", (2 * N, 2 * N * DIN))

        mo = big.tile([DIN, L], F16, tag="mo")   # g rows 0:64, x rows 64:128
        mo_sw = big.tile([DIN, L], F16, tag="mo_sw")  # swapped halves

        # ================= Phase M =================
        from contextlib import ExitStack as _ES
        mctx = _ES()
        mpS = mctx.enter_context(tc.tile_pool(name="mpS", bufs=1))
        mpT = mctx.enter_context(tc.tile_pool(name="mpT", bufs=1))
        mpT2 = mctx.enter_context(tc.tile_pool(name="mpT2", bufs=2))
        pp = mctx.enter_context(tc.tile_pool(name="pp", bufs=4, space="PSUM"))
        bcp = mctx.enter_context(tc.tile_pool(name="bcp", bufs=2, space="PSUM"))

        st = [{}, {}]   # per-job steady tiles

        def prep(j):
            s_ = st[j]
            xs = mpS.tile([DIN, 3 + L], F16, tag=f"xs{j}")
            nc.vector.memset(xs[:, 0:3], 0.0)
            gate = mpS.tile([DIN, L], F16, tag=f"gate{j}")
            seqt = mpS.tile([C, L], F16, tag=f"seqt{j}")
            nc.sync.dma_start(seqt[:], di["seq_g" if j == 0 else "seq_x"].ap())
            for cc in range(NCH):
                s = seqt[:, cc * LC:(cc + 1) * LC]
                pxs = pp.tile([DIN, LC], F32, tag="pmm")
                nc.tensor.matmul(pxs[:], inwT[:, j * 2 * DIN:j * 2 * DIN + DIN],
                                 s, start=True, stop=True)
                nc.scalar.copy(xs[:, 3 + cc * LC:3 + (cc + 1) * LC], pxs[:])
                pz = pp.tile([DIN, LC], F32, tag="pmm")
                nc.tensor.matmul(pz[:],
                                 inwT[:, j * 2 * DIN + DIN:(j + 1) * 2 * DIN],
                                 s, start=True, stop=True)
                nc.scalar.activation(gate[:, cc * LC:(cc + 1) * LC], pz[:],
                                     AF.Silu)
            xc = mpS.tile([DIN, L], F16, tag=f"xc{j}")
            for cc in range(NCH):
                pxc = pp.tile([DIN, LC], F32, tag="pmm")
                for k in range(DCONV):
                    nc.tensor.matmul(
                        pxc[:],
                        cwdiag[:, (j * DCONV + k) * DIN:(j * DCONV + k + 1) * DIN],
                        xs[:, cc * LC + k:cc * LC + k + LC],
                        start=(k == 0), stop=(k == DCONV - 1))
                nc.scalar.activation(xc[:, cc * LC:(cc + 1) * LC], pxc[:],
                                     AF.Silu, bias=cb[:, j:j + 1])
            dbl = mpS.tile([E, L], F16, tag=f"dbl{j}")
            for cc in range(NCH):
                pdb = pp.tile([E, LC], F32, tag="pmm")
                nc.tensor.matmul(pdb[:], xpwT[:, j * E:(j + 1) * E],
                                 xc[:, cc * LC:(cc + 1) * LC],
                                 start=True, stop=True)
                nc.vector.tensor_copy(dbl[:, cc * LC:(cc + 1) * LC], pdb[:])
            dt = mpS.tile([DIN, L], F16, tag=f"dt{j}")
            for cc in range(NCH):
                pdt = pp.tile([DIN, LC], F32, tag="pmm")
                nc.tensor.matmul(pdt[:],
                                 dtw36[2 * N:2 * N + DTR, j * DIN:(j + 1) * DIN],
                                 dbl[2 * N:2 * N + DTR, cc * LC:(cc + 1) * LC],
                                 start=True, stop=True)
                etmp = mpS.tile([DIN, LC], F32, tag="etmp")
                nc.scalar.activation(etmp[:], pdt[:], AF.Exp,
                                     bias=dtb[:, j:j + 1])
                nc.scalar.activation(dt[:, cc * LC:(cc + 1) * LC], etmp[:],
                                     AF.Ln, bias=1.0)
            u = mpS.tile([DIN, L], F16, tag=f"u{j}")
            nc.vector.scalar_tensor_tensor(u[:], dt[:], USCALE, xc[:],
                                           ALU.mult, ALU.mult)
            dA1 = mpS.tile([DIN, L], F16, tag=f"dA1{j}")
            nc.scalar.activation(dA1[:], dt[:], AF.Exp,
                                 scale=Acol[:, j * N:j * N + 1])
            s_.update(gate=gate, xc=xc, dbl=dbl, dt=dt, u=u, dA1=dA1)

        ev_counter = [0]

        def evict(dst, src):
            e = EV_PATTERN[ev_counter[0] % len(EV_PATTERN)]
            ev_counter[0] += 1
            if e == "A":
                nc.scalar.copy(dst, src)
            else:
                nc.vector.tensor_copy(dst, src)

        def scan_loop(j):
            s_ = st[j]
            dbl, dt, u, dA1 = s_["dbl"], s_["dt"], s_["u"], s_["dA1"]
            y = mpS.tile([DIN, L], F16, tag=f"xs{j}")       # reuse xs slot
            yg = mpS.tile([DIN, L], F16, tag=f"seqt{j}")    # reuse seqt slot
            carry = mpS.tile([DIN, N], F16, tag=f"carry{j}")
            for hf in range(2):
                h0 = hf * LH
                prev_dA = None
                for n in range(N):
                    Bbc = mpT2.tile([DIN, LH], F16, tag="Bbc")
                    Cbc = mpT2.tile([DIN, LH], F16, tag="Cbc")
                    for q in range(LH // BCP):
                        pb = bcp.tile([DIN, BCP], F32, tag="pbc")
                        for h2 in range(BCP // LC):
                            sl = slice(h0 + q * BCP + h2 * LC,
                                       h0 + q * BCP + (h2 + 1) * LC)
                            nc.tensor.matmul(pb[:, h2 * LC:(h2 + 1) * LC],
                                             onehotBC[:, n * DIN:(n + 1) * DIN],
                                             dbl[0:2 * N, sl],
                                             start=True, stop=True)
                        evict(Bbc[:, q * BCP:(q + 1) * BCP], pb[:])
                        pc = bcp.tile([DIN, BCP], F32, tag="pbc")
                        for h2 in range(BCP // LC):
                            sl = slice(h0 + q * BCP + h2 * LC,
                                       h0 + q * BCP + (h2 + 1) * LC)
                            nc.tensor.matmul(
                                pc[:, h2 * LC:(h2 + 1) * LC],
                                onehotBC[:, (N + n) * DIN:(N + n + 1) * DIN],
                                dbl[0:2 * N, sl], start=True, stop=True)
                        evict(Cbc[:, q * BCP:(q + 1) * BCP], pc[:])
                    if n == 0:
                        dA = dA1[:, h0:h0 + LH]
                    elif n % 2 == 0 or not DA_GPS:
                        dAt = mpT2.tile([DIN, LH], F16, tag="dA")
                        nc.scalar.activation(
                            dAt[:], dt[:, h0:h0 + LH], AF.Exp,
                            scale=Acol[:, j * N + n:j * N + n + 1])
                        dA = dAt[:]
                    else:
                        dAt = mpT2.tile([DIN, LH], F16, tag="dA")
                        nc.gpsimd.tensor_mul(dAt[:], prev_dA, dA1[:, h0:h0 + LH])
                        dA = dAt[:]
                    prev_dA = dA
                    dBx = mpT.tile([DIN, LH], F16, tag="dBx")
                    nc.vector.tensor_mul(dBx[:], u[:, h0:h0 + LH], Bbc[:])
                    hsc = mpT.tile([DIN, LH], F16, tag="hsc")
                    init = 0.0 if hf == 0 else carry[:, n:n + 1]
                    nc.vector.tensor_tensor_scan(hsc[:], dA, dBx[:], init,
                                                 ALU.mult, ALU.add)
                    if hf == 0:
                        nc.vector.tensor_copy(carry[:, n:n + 1],
                                              hsc[:, LH - 1:LH])
                    if n == 0:
                        nc.vector.tensor_mul(y[:, h0:h0 + LH], hsc[:], Cbc[:])
                    elif n == 1:
                        nc.vector.tensor_mul(yg[:, h0:h0 + LH], hsc[:], Cbc[:])
                    elif ACC_DVE_EVERY <= 1 or n % ACC_DVE_EVERY == ACC_DVE_EVERY - 1:
                        prod = mpT.tile([DIN, LH], F16, tag="prod")
                        nc.vector.tensor_mul(prod[:], hsc[:], Cbc[:])
                        nc.vector.tensor_add(y[:, h0:h0 + LH],
                                             y[:, h0:h0 + LH], prod[:])
                    else:
                        prod = mpT2.tile([DIN, LH], F16, tag="prodg")
                        nc.vector.tensor_mul(prod[:], hsc[:], Cbc[:])
                        nc.gpsimd.tensor_add(yg[:, h0:h0 + LH],
                                             yg[:, h0:h0 + LH], prod[:])
            nc.vector.tensor_add(y[:], y[:], yg[:])
            return y

        def tail(j, y):
            s_ = st[j]
            yfull = mpS.tile([DIN, L], F16, tag=f"u{j}")    # reuse u slot
            nc.vector.tensor_scalar_mul(yfull[:], s_["xc"][:], Dpw[:, j:j + 1])
            nc.vector.scalar_tensor_tensor(yfull[:], y[:], 1.0 / USCALE,
                                           yfull[:], ALU.mult, ALU.add)
            nc.vector.tensor_mul(yfull[:], yfull[:], s_["gate"][:])
            for cc in range(NCH):
                pm = pp.tile([DIN, LC], F32, tag="pmm")
                lo, hi = (0, C) if j == 0 else (C, DIN)
                slo, shi = (C, DIN) if j == 0 else (0, C)
                nc.tensor.matmul(pm[lo:hi, :], owT[:, j * C:(j + 1) * C],
                                 yfull[:, cc * LC:(cc + 1) * LC],
                                 start=True, stop=True)
                nc.tensor.matmul(pm[slo:shi, :], owT[:, j * C:(j + 1) * C],
                                 yfull[:, cc * LC:(cc + 1) * LC],
                                 start=True, stop=True)
                nc.vector.tensor_copy(
                    mo[lo:hi, cc * LC:(cc + 1) * LC], pm[lo:hi, :])
                nc.vector.tensor_copy(
                    mo_sw[slo:shi, cc * LC:(cc + 1) * LC], pm[slo:shi, :])

        for _rep in range(reps):
            prep(0)
            prep(1)
            y0 = scan_loop(0)
            tail(0, y0)
            y1 = scan_loop(1)
            tail(1, y1)
        mctx.close()

        # ================= Phase B =================
        PW = WW + 2
        RPP = LC // WW
        import os as _os2
        pb_reps = reps if _os2.environ.get("K_PBREPS", "0") == "1" else 1
        for _pbr in range(pb_reps):
          with tc.tile_pool(name="bp", bufs=1) as bp, \
                  tc.tile_pool(name="bps", bufs=4, space="PSUM") as bps, \
                  tc.tile_pool(name="dram", bufs=1, space="DRAM") as dram:
              dwdiag = wload(bp, "dwdiag", (DIN, 9 * C))
              dwbias = wload(bp, "dwbias", (DIN, 1), F32)
              pdwdiag = wload(bp, "pdwdiag", (DIN, 9 * C))
              pdwbias = wload(bp, "pdwbias", (C, 1), F32)
              p1wT = wload(bp, "p1wT", (C, FINT))
              p1bias = wload(bp, "p1bias", (FINT, 1), F32)
              c3stat = wload(bp, "c3stat", (C, 9 * FINT))
              c3bias = wload(bp, "c3bias", (FINT, 1), F32)
              c1w = wload(bp, "c1w", (FINT, 1))
              c1bias = wload(bp, "c1bias", (1, 1), F32)
              xsl = bp.tile([C, L], F32, tag="xsl")
              nc.sync.dma_start(xsl[:], di["xslice"].ap())

              # dw conv split into g (rows 0:64, runs right after job-0 tail)
              # and x (rows 64:128, after job-1) on separate pad tiles
              pad_g = bp.tile([C, PW * PW], F16, tag="pad_g")
              nc.vector.memset(pad_g[:], 0.0)
              padg_v = pad_g[:].rearrange("p (h w) -> p h w", h=PW, w=PW)
              nc.vector.tensor_copy(padg_v[:, 1:1 + HH, 1:1 + WW],
                                    mo[0:C, :].rearrange("p (h w) -> p h w",
                                                         h=HH, w=WW))
              pad_x = bp.tile([DIN, PW * PW], F16, tag="pad_x")
              nc.vector.memset(pad_x[C:DIN, :], 0.0)
              padx_v = pad_x[:].rearrange("p (h w) -> p h w", h=PW, w=PW)
              nc.vector.tensor_copy(padx_v[C:DIN, 1:1 + HH, 1:1 + WW],
                                    mo[C:DIN, :].rearrange("p (h w) -> p h w",
                                                           h=HH, w=WW))
              sg = bp.tile([DIN, L], F16, tag="sg")
              gcb = bp.tile([DIN, L], F16, tag="gcb")
              for cc in range(NCH):
                  pcv = bps.tile([C, LC], F32, tag="pbm")
                  for t in range(9):
                      ty, tx = t // 3, t % 3
                      mv = padg_v[:, ty + cc * RPP:ty + cc * RPP + RPP, tx:tx + WW]
                      nc.tensor.matmul(pcv[:], dwdiag[0:C, t * C:(t + 1) * C],
                                       mv, start=(t == 0), stop=(t == 8))
                  nc.scalar.activation(gcb[0:C, cc * LC:(cc + 1) * LC], pcv[:],
                                       AF.Relu, bias=dwbias[0:C, :])
              for cc in range(NCH):
                  pcx = bps.tile([DIN, LC], F32, tag="pbx")
                  mvx = padx_v[C:DIN, 0:1, 0:1]
                  for t in range(9):
                      ty, tx = t // 3, t % 3
                      mv = padx_v[C:DIN, ty + cc * RPP:ty + cc * RPP + RPP,
                                  tx:tx + WW]
                      nc.tensor.matmul(pcx[C:DIN, :],
                                       dwdiag[C:DIN, t * C:(t + 1) * C],
                                       mv, start=(t == 0), stop=(t == 8))
                  nc.scalar.activation(gcb[C:DIN, cc * LC:(cc + 1) * LC],
                                       pcx[C:DIN, :], AF.Relu,
                                       bias=dwbias[C:DIN, :])
              for hh in range(4):
                  q = slice(hh * 1024, (hh + 1) * 1024)
                  nc.scalar.activation(sg[:, q], gcb[:, q], AF.Sigmoid)
              # mo_sw holds the partition-swapped mamba outputs, so the
              # cross-products come from one multiply; the pdw conv's stacked
              # diagonal is symmetric in the two halves, so the swapped order
              # yields the identical cross-sum.
              prodb = bp.tile([DIN, L], F16, tag="prodb")
              for hh in range(2):
                  q = slice(hh * LH, (hh + 1) * LH)
                  nc.vector.tensor_mul(prodb[:, q], mo_sw[:, q], sg[:, q])
              padc = bp.tile([DIN, PW * PW], F16, tag="padc")
              nc.vector.memset(padc[:], 0.0)
              padc_v = padc[:].rearrange("p (h w) -> p h w", h=PW, w=PW)
              nc.vector.tensor_copy(
                  padc_v[:, 1:1 + HH, 1:1 + WW],
                  prodb[:].rearrange("p (h w) -> p h w", h=HH, w=WW))
              h1 = bp.tile([C, L], F16, tag="h1")
              for cc in range(NCH):
                  pcv2 = bps.tile([C, LC], F32, tag="pbm")
                  for t in range(9):
                      ty, tx = t // 3, t % 3
                      mv = padc_v[:, ty + cc * RPP:ty + cc * RPP + RPP, tx:tx + WW]
                      nc.tensor.matmul(pcv2[:], pdwdiag[:, t * C:(t + 1) * C],
                                       mv, start=(t == 0), stop=(t == 8))
                  nc.scalar.activation(h1[:, cc * LC:(cc + 1) * LC], pcv2[:],
                                       AF.Relu, bias=pdwbias[:])
              projs = bp.tile([FINT, L], F16, tag="projs")
              for cc in range(NCH):
                  pp1 = bps.tile([FINT, LC], F32, tag="pbm")
                  nc.tensor.matmul(pp1[:], p1wT[:], h1[:, cc * LC:(cc + 1) * LC],
                                   start=True, stop=True)
                  nc.scalar.activation(projs[:, cc * LC:(cc + 1) * LC], pp1[:],
                                       AF.Relu, bias=p1bias[:])
              padp = bp.tile([FINT, PW * PW], F16, tag="padp")
              nc.vector.memset(padp[:], 0.0)
              padp_v = padp[:].rearrange("p (h w) -> p h w", h=PW, w=PW)
              nc.vector.tensor_copy(
                  padp_v[:, 1:1 + HH, 1:1 + WW],
                  projs[:].rearrange("p (h w) -> p h w", h=HH, w=WW))
              part = bp.tile([FINT, L], F32, tag="part")
              cin = dram.tile([FINT, L], F32, tag="cin")
              cout = dram.tile([FINT, L], F32, tag="cout")
              for cc in range(NCH):
                  pc3 = bps.tile([FINT, LC], F32, tag="pbm")
                  for t in range(9):
                      ty, tx = t // 3, t % 3
                      mv = padp_v[:, ty + cc * RPP:ty + cc * RPP + RPP, tx:tx + WW]
                      nc.tensor.matmul(pc3[:], c3stat[:, t * FINT:(t + 1) * FINT],
                                       mv, start=(t == 0), stop=(t == 8))
                  nc.vector.tensor_copy(part[:, cc * LC:(cc + 1) * LC], pc3[:])
                  nc.sync.dma_start(cin[:, cc * LC:(cc + 1) * LC],
                                    part[:, cc * LC:(cc + 1) * LC])
              if collective:
                  nc.gpsimd.collective_compute(
                      "AllReduce", ALU.add,
                      replica_groups=[[0, 1, 2, 3], [4, 5, 6, 7]],
                      ins=[cin.opt()], outs=[cout.opt()])
              else:
                  nc.sync.dma_start(cout[:], cin[:])
              h3 = bp.tile([FINT, L], F32, tag="h3")
              nc.sync.dma_start(h3[:], cout[:])
              hf_ = bp.tile([FINT, L], F16, tag="hf")
              for hh in range(2):
                  q = slice(hh * LH, (hh + 1) * LH)
                  nc.scalar.activation(hf_[:, q], h3[:, q], AF.Relu,
                                       bias=c3bias[:])
              psi = bp.tile([1, L], F16, tag="psi")
              outt = bp.tile([C, L], F32, tag="outt")
              for cc in range(NCH):
                  pps = bps.tile([1, LC], F32, tag="pbm")
                  nc.tensor.matmul(pps[:], c1w[:], hf_[:, cc * LC:(cc + 1) * LC],
                                   start=True, stop=True)
                  nc.scalar.activation(psi[:, cc * LC:(cc + 1) * LC], pps[:],
                                       AF.Sigmoid, bias=c1bias[:])
              for cc in range(NCH):
                  pbc2 = bps.tile([C, LC], F32, tag="pbm")
                  nc.tensor.matmul(pbc2[:], ones1[0:1, 0:C],
                                   psi[:, cc * LC:(cc + 1) * LC],
                                   start=True, stop=True)
                  nc.vector.tensor_mul(outt[:, cc * LC:(cc + 1) * LC],
                                       xsl[:, cc * LC:(cc + 1) * LC], pbc2[:])
              nc.sync.dma_start(out_d.ap(), outt[:])

    return nc


def _legalize_bir_waits(bir_bytes):
    """Walrus here allows 1 sync-wait per instruction (2 for EventSemaphore);
    Tile emits more. Hoist extras onto inserted EventSemaphore carriers."""
    import orjson
    bir = orjson.loads(bir_bytes)
    for fn in bir.get("functions", []):
        for blk in fn.get("blocks", []):
            ins_list = blk.get("instructions")
            if not ins_list:
                continue
            out = []
            for ins in ins_list:
                si = ins.get("sync_info")
                waits = (si or {}).get("on_wait") or []
                cap = 2 if ins.get("opcode") == "EventSemaphore" else 1
                if len(waits) > cap:
                    extra, keep = waits[:-cap], waits[-cap:]
                    for i in range(0, len(extra), 2):
                        out.append({
                            "debug": ins.get("debug", 0),
                            "engine": ins["engine"], "ins": [],
                            "name": f"{ins['name']}_wfix{i}",
                            "opcode": "EventSemaphore", "outs": [],
                            "sync_info": {"on_update": [],
                                          "on_wait": extra[i:i + 2]},
                        })
                    si["on_wait"] = keep
                out.append(ins)
            blk["instructions"] = out
    return orjson.dumps(bir)


def _get_compiled():
    if "nc" not in _COMPILED:
        nc = _build()
        orig = nc.to_json_bytes
        nc.to_json_bytes = lambda: _legalize_bir_waits(orig())
        _COMPILED["nc"] = nc
    return _COMPILED["nc"]


def _prep_inputs(c, inputs):
    """Host-side prep for core c (branch i = c%4, batch b = c//4)."""
    i, b = c % 4, c // 4
    f16, f32 = np.float16, np.float32
    g, x = np.asarray(inputs["g"]), np.asarray(inputs["x"])
    sl = slice(i * C, (i + 1) * C)
    m = {}
    m["seq_g"] = g[b, sl].reshape(C, L).astype(f16)
    m["seq_x"] = x[b, sl].reshape(C, L).astype(f16)
    m["xslice"] = x[b, sl].reshape(C, L).astype(f32)
    layers = (i, 4 + i)
    inw = np.asarray(inputs["inw"]); cw = np.asarray(inputs["cw"])
    cbv = np.asarray(inputs["cb"]); xpw = np.asarray(inputs["xpw"])
    dtw = np.asarray(inputs["dtw"]); dtbv = np.asarray(inputs["dtb"])
    Alog = np.asarray(inputs["Alog"]); Dpv = np.asarray(inputs["Dp"])
    ow = np.asarray(inputs["ow"])
    m["inwT"] = np.concatenate([inw[j].T for j in layers], axis=1).astype(f16)
    cwd = np.zeros((2, DCONV, DIN, DIN), f32)
    for a, j in enumerate(layers):
        for k in range(DCONV):
            np.fill_diagonal(cwd[a, k], cw[j, :, k])
    m["cwdiag"] = cwd.transpose(2, 0, 1, 3).reshape(DIN, 2 * DCONV * DIN).astype(f16)
    m["cb"] = np.stack([cbv[j] for j in layers], axis=1).astype(f32)
    perm = list(range(DTR, DTR + 2 * N)) + list(range(DTR))
    m["xpwT"] = np.concatenate([xpw[j][perm].T for j in layers], axis=1).astype(f16)
    m["dtwT"] = np.concatenate([dtw[j].T for j in layers], axis=1).astype(f16)
    m["dtb"] = np.stack([dtbv[j] for j in layers], axis=1).astype(f32)
    m["Acol"] = np.concatenate([-np.exp(Alog[j]) for j in layers], axis=1).astype(f32)
    m["Dp"] = np.stack([Dpv[j] for j in layers], axis=1).astype(f32)
    m["owT"] = np.concatenate([ow[j].T for j in layers], axis=1).astype(f16)
    m["ones1"] = np.ones((1, DIN), f16)
    ohbc = np.zeros((2 * N, 2 * N * DIN), f16)
    for r in range(2 * N):
        ohbc[r, r * DIN:(r + 1) * DIN] = 1.0
    m["onehotBC"] = ohbc
    dwg_w = np.asarray(inputs["dwg_w"])[i]; dwg_s = np.asarray(inputs["dwg_s"])[i]
    dwx_w = np.asarray(inputs["dwx_w"])[i]; dwx_s = np.asarray(inputs["dwx_s"])[i]
    dwg = dwg_w * dwg_s[:, None, None]; dwx = dwx_w * dwx_s[:, None, None]
    dwd = np.zeros((9, DIN, C), f32)
    for t in range(9):
        ty, tx = t // 3, t % 3
        np.fill_diagonal(dwd[t, 0:C, :], dwg[:, ty, tx])
        np.fill_diagonal(dwd[t, C:DIN, :], dwx[:, ty, tx])
    m["dwdiag"] = dwd.transpose(1, 0, 2).reshape(DIN, 9 * C).astype(f16)
    dwb = np.concatenate([
        np.asarray(inputs["dwg_b"])[i] * dwg_s + np.asarray(inputs["dwg_t"])[i],
        np.asarray(inputs["dwx_b"])[i] * dwx_s + np.asarray(inputs["dwx_t"])[i]])
    m["dwbias"] = dwb.reshape(DIN, 1).astype(f32)
    pdw_w = np.asarray(inputs["pdw_w"])[i]; pdw_s = np.asarray(inputs["pdw_s"])[i]
    pdw = pdw_w * pdw_s[:, None, None]
    pdd = np.zeros((9, DIN, C), f32)
    for t in range(9):
        np.fill_diagonal(pdd[t, 0:C], pdw[:, t // 3, t % 3])
        np.fill_diagonal(pdd[t, C:DIN], pdw[:, t // 3, t % 3])
    m["pdwdiag"] = pdd.transpose(1, 0, 2).reshape(DIN, 9 * C).astype(f16)
    m["pdwbias"] = (np.asarray(inputs["pdw_b"])[i] * pdw_s
                    + np.asarray(inputs["pdw_t"])[i]).reshape(C, 1).astype(f32)
    p1_w = np.asarray(inputs["p1_w"])[i]; p1_s = np.asarray(inputs["p1_s"])[i]
    m["p1wT"] = (p1_w * p1_s[:, None]).T.astype(f16)
    m["p1bias"] = (np.asarray(inputs["p1_b"])[i] * p1_s
                   + np.asarray(inputs["p1_t"])[i]).reshape(FINT, 1).astype(f32)
    c3_w = np.asarray(inputs["c3_w"]); c3_s = np.asarray(inputs["c3_s"])
    c3st = np.zeros((9, C, FINT), f32)
    for t in range(9):
        ty, tx = t // 3, t % 3
        c3st[t] = (c3_w[:, i * C:(i + 1) * C, ty, tx] * c3_s[:, None]).T
    m["c3stat"] = c3st.transpose(1, 0, 2).reshape(C, 9 * FINT).astype(f16)
    m["c3bias"] = (np.asarray(inputs["c3_b"]) * c3_s
                   + np.asarray(inputs["c3_t"])).reshape(FINT, 1).astype(f32)
    c1_w = np.asarray(inputs["c1_w"]); c1_s = np.asarray(inputs["c1_s"])
    m["c1w"] = (c1_w[0] * c1_s[0]).reshape(FINT, 1).astype(f16)
    m["c1bias"] = np.asarray(
        np.asarray(inputs["c1_b"])[0] * c1_s[0] + np.asarray(inputs["c1_t"])[0],
        dtype=f32).reshape(1, 1)
    return m


def kernel(**inputs):
    from concourse import bass_utils
    nc = _get_compiled()
    in_maps = [_prep_inputs(c, inputs) for c in range(8)]
    res = bass_utils.run_bass_kernel_spmd(nc, in_maps, core_ids=list(range(8)))
    out = np.empty((B, NB * C, HH, WW), np.float32)
    for c in range(8):
        i, b = c % 4, c // 4
        out[b, i * C:(i + 1) * C] = res.results[c]["outsl"].reshape(C, HH, WW)
    return out

